# revision 47
# baseline (speedup 1.0000x reference)
"""Trainium2 Bass kernel for nn_Attention (LN -> QKV -> softmax attn -> out proj).

Sharding: 8 cores; core c handles batch b=c//4 and heads [4*(c%4), 4*(c%4)+4).
Each core computes one partial output [DIM, T] (both head-pair stacks summed
in PSUM); the host sums 4 partials per batch, transposes, adds b_out.

Device pipeline per core (bf16 matmuls, fp32 PSUM accumulate):
  Phase 1 (per 512-token slab): K/Q/V projections from raw x^T with the
    LayerNorm mean correction folded into the weights on the host
    (W'' = W' - colsum(W')/DIM, so x @ W'' == (x - mu) @ W'), plus:
      - Sx rides the V matmul as an appended ones-column in the moving
        operand (psum col CV = row-sum of x) -- no separate stats matmul.
      - Sxx via one DVE square + 1^T xsq matmuls; rsqrt via bit-trick
        Newton on DVE in token-column layout.
      - q scaled by r on psum->sbuf move (fused DVE multiply with a
        DMA-broadcast r row); k left unscaled (r_key rides the exp scale);
        v scaled by r via per-partition tensor_scalar.
  Phase 2 attention, per (512-query chunk, head pair): the two heads of a
    pair live on PE row-groups 0:64 / 64:128, so their K=64 score matmuls
    run CONCURRENTLY on disjoint quadrants (auto tile_position).  exp is
    split: ScalarE does SPL/512 of each tile exactly; DVE does the rest
    with a one-op bf16 Schraudolph (int16 bits = s*C1 + C2, bitcast).
    P@V accumulates with a ones-column appended to v so the softmax
    denominator lands in the same PSUM tile; denominator reciprocal via
    DMA-reshape + fast-reciprocal, broadcast back through DRAM.
  Out-proj: both stacks accumulate into ONE psum tile per (oc, chunk);
    units are interleaved into the next chunk's attention loop.
"""

import contextlib

import numpy as np

import concourse.bass as bass
import concourse.tile as tile
from concourse import bacc, mybir
from concourse import bass_utils

# Problem constants (hardcoded per contract)
B, N, DIM = 2, 2048, 1024
H, DH = 16, 64
INNER = H * DH
LN_EPS = 1e-5
SCALE = DH ** -0.5

# Per-core constants
P = 128
T = N                 # tokens per core (one batch)
TT = T // P           # 16 token tiles of 128
NT4 = T // 512        # 4 token slabs of 512
KD = DIM // P         # 8 contraction tiles
HL = 4                # local heads per core
CQK = 2 * HL * DH     # 512 (q cols + k cols)
CV = HL * DH          # 256 (v cols)
CV1 = CV + 1          # v cols + ones column (Sx)
KT = T // P           # 16 key tiles of 128
NQ = T // 512         # 4 query chunks of 512

# exp split: ScalarE does cols [0, SPL), DVE-Schraudolph does [SPL, 512)
SPL = 296
# bf16 Schraudolph constants: int16 bits = s * C1 + C2, bitcast to bf16
C1_SCH = float(128.0 * np.log2(np.e))
C2_SCH = float(16256.0 + 128.0 * (-0.043))

f32 = mybir.dt.float32
f32r = mybir.dt.float32r
bf16 = mybir.dt.bfloat16
i16 = mybir.dt.int16
i32 = mybir.dt.int32
FT = mybir.ActivationFunctionType
ALU = mybir.AluOpType

import ml_dtypes
_BF16 = np.dtype(ml_dtypes.bfloat16)

_CACHE = {}


def _build():
    nc = bacc.Bacc("TRN2", target_bir_lowering=False, debug=False)

    xt_d = nc.dram_tensor("xt", [DIM, T], bf16, kind="ExternalInput").ap()
    wqk_d = nc.dram_tensor("wqk", [DIM, CQK], bf16, kind="ExternalInput").ap()
    wv1_d = nc.dram_tensor("wv1", [DIM, CV1], bf16, kind="ExternalInput").ap()
    wout_d = nc.dram_tensor("wout", [2 * P, DIM], f32r, kind="ExternalInput").ap()
    outp_d = nc.dram_tensor("outp", [DIM, T], f32, kind="ExternalOutput").ap()

    with tile.TileContext(nc) as tc, contextlib.ExitStack() as ctx:
        pers = ctx.enter_context(tc.tile_pool(name="pers", bufs=1))
        dram = ctx.enter_context(tc.tile_pool(name="dram", bufs=1, space="DRAM"))

        # per-slab tiles: dependency tracking is per-tile, so slab-sliced
        # tiles keep attention from falsely waiting on the last slab's write
        qTs = [pers.tile([P, 2, 512], bf16, name=f"qT{t}") for t in range(NT4)]
        kTs = [pers.tile([P, 2, 512], bf16, name=f"kT{t}") for t in range(NT4)]
        vaugs = [pers.tile([P, 4, HL, DH + 1], bf16, name=f"vaug{t}")
                 for t in range(NT4)]
        outT = pers.tile([P, 2, T], f32r)            # attention out (pair-stacked)
        wout_sb = pers.tile([P, 2, DIM], f32r)
        r_c = pers.tile([P, TT], f32)                # rsqrt(var+eps), token cols
        rc1 = pers.tile([P, TT], f32)                # r * C1_SCH for DVE exp
        r_bc = pers.tile([P, T], f32)                # r broadcast along partitions
        onesb = pers.tile([1, P], bf16)              # denominator-broadcast lhsT
        mu_c = pers.tile([P, TT], f32)
        sxxc = pers.tile([P, TT], f32)
        ex2e = pers.tile([P, TT], f32)
        mu2 = pers.tile([P, TT], f32)
        vvar = pers.tile([P, TT], f32)
        yint = pers.tile([P, TT], i32)
        tnw = pers.tile([P, TT], f32)
        magic = pers.tile([P, TT], i32)
        sxx_row = pers.tile([1, T], f32)

        sxx_dram = dram.tile([1, T], f32)
        r_dram = dram.tile([1, T], f32)
        dnm_dram = dram.tile([2, 2, T], f32)         # [pair, head, tok]
        rdn_dram = dram.tile([2, 2, T], bf16)

        for t in range(NT4):
            nc.vector.memset(vaugs[t][:], 1.0)
        nc.vector.memset(magic[:], 0x5F3759DF)
        nc.vector.memset(onesb[:], 1.0)

        # ---------------- Phase 1: stats + QKV projection -----------------
        with tc.tile_pool(name="pab", bufs=1) as pab, \
             tc.tile_pool(name="pxt", bufs=3) as pxt, \
             tc.tile_pool(name="pxq", bufs=2) as pxq, \
             tc.tile_pool(name="pvr", bufs=2) as pvr, \
             tc.tile_pool(name="pgk", bufs=3, space="PSUM") as pgk, \
             tc.tile_pool(name="pgv", bufs=4, space="PSUM") as pgv, \
             tc.tile_pool(name="pgs", bufs=1, space="PSUM") as pgs:

            def load_xt(t4):
                # one 3D DMA per slab: per-dma_start Sync issue cost is
                # ~600ns, so 8 separate kt loads serialize the Sync queue
                tsl = slice(t4 * 512, (t4 + 1) * 512)
                xt_t = pxt.tile([P, KD, 512], bf16, tag="xt", name=f"xt{t4}")
                nc.sync.dma_start(
                    xt_t[:], xt_d[:, tsl].rearrange("(o p) t -> p o t", p=P))
                return xt_t

            wqk_sb = pab.tile([P, KD, CQK], bf16)
            # k half first: the first matmuls need it
            nc.sync.dma_start(
                wqk_sb[:, :, CQK // 2:],
                wqk_d[:, CQK // 2:].rearrange("(o p) c -> p o c", p=P))
            xt_tiles = {0: load_xt(0)}
            wv1_sb = pab.tile([P, KD, CV1], bf16)
            nc.sync.dma_start(wv1_sb[:], wv1_d.rearrange("(o p) c -> p o c", p=P))
            nc.sync.dma_start(
                wqk_sb[:, :, :CQK // 2],
                wqk_d[:, :CQK // 2].rearrange("(o p) c -> p o c", p=P))
            nc.sync.dma_start(wout_sb[:], wout_d.rearrange("(o p) c -> p o c", p=P))

            ones_col = pab.tile([P, 1], bf16)
            nc.vector.memset(ones_col[:], 1.0)

            # PE warmup spins during initial DMA fill (HAM un-throttle)
            bones = pab.tile([1, 1], bf16)
            nc.vector.memset(bones[:], 1.0)
            brow = pab.tile([1, 64], bf16)
            nc.vector.memset(brow[:], 1.0)
            warm_ps = pgs.tile([1, 512], f32, tag="st", name="warm0")
            for _ in range(60):
                nc.tensor.matmul(warm_ps[0:1, 0:64], bones[:], brow[:],
                                 start=True, stop=True)

            def emit_xsq(t4):
                # squares for Sxx (DVE, bf16 2x rate)
                xsq = pxq.tile([P, KD, 512], bf16, tag="xsq",
                               name=f"xsq{t4}")
                nc.vector.tensor_tensor(xsq[:], xt_tiles[t4][:],
                                        xt_tiles[t4][:], ALU.mult)
                return xsq

            def emit_q_mm(t4):
                # Q projection matmuls for slab t4 (one slab behind K/V:
                # PE work to cover this slab's stats chain)
                xt_t = xt_tiles.pop(t4)
                pss = []
                for g in (0, 1):
                    ps = pgk.tile([P, 512], f32, tag="qk", name=f"q{t4}_{g}")
                    for kt in range(KD):
                        nc.tensor.matmul(ps[:], wqk_sb[:, kt, g * P:(g + 1) * P],
                                         xt_t[:, kt],
                                         start=(kt == 0), stop=(kt == KD - 1))
                    pss.append(ps)
                return pss

            def emit_q_fold(t4, pss):
                # r folded into the psum->sbuf move; emitted after this
                # slab's Newton so the DVE queue head never waits on the
                # r_bc DMA chain
                tsl = slice(t4 * 512, (t4 + 1) * 512)
                for g in (0, 1):
                    nc.vector.tensor_tensor(qTs[t4][:, g, :], pss[g][:],
                                            r_bc[:, tsl], ALU.mult)

            xsq_tiles = {0: emit_xsq(0)}

            for t4 in range(NT4):
                tsl = slice(t4 * 512, (t4 + 1) * 512)
                s4 = slice(t4 * 4, t4 * 4 + 4)
                if t4 + 1 < NT4:
                    xt_tiles[t4 + 1] = load_xt(t4 + 1)
                xt_t = xt_tiles[t4]
                xsq = xsq_tiles.pop(t4)

                # previous slab's Q at the head of the iteration: the fold
                # sits FIRST in the DVE queue (its r_bc landed a slab ago),
                # ahead of this slab's DMA-gated Newton chain
                if t4 >= 1:
                    emit_q_fold(t4 - 1, emit_q_mm(t4 - 1))

                # K projections (k stays un-scaled: r_key rides the exp scale)
                for g in (2, 3):
                    ps = pgk.tile([P, 512], f32, tag="qk", name=f"k{t4}_{g}")
                    for kt in range(KD):
                        nc.tensor.matmul(ps[:], wqk_sb[:, kt, g * P:(g + 1) * P],
                                         xt_t[:, kt],
                                         start=(kt == 0), stop=(kt == KD - 1))
                    nc.scalar.copy(kTs[t4][:, g - 2, :], ps[:])

                # Sxx row = 1^T xsq
                ps_st = pgs.tile([1, 512], f32, tag="st", name=f"st{t4}")
                for kt in range(KD):
                    nc.tensor.matmul(ps_st[0:1], ones_col[:], xsq[:, kt],
                                     start=(kt == 0), stop=(kt == KD - 1))
                nc.scalar.copy(sxx_row[0:1, tsl], ps_st[0:1])
                nc.sync.dma_start(sxx_dram[0:1, tsl], sxx_row[0:1, tsl])
                nc.sync.dma_start(
                    sxxc[:, s4],
                    sxx_dram[0, tsl].rearrange("(o p) -> p o", p=P))

                # V projections + Sx ones-column (psum col CV); staged
                # into SBUF via ACT so the psum bank frees immediately
                vraw = pvr.tile([P, 4, CV1], f32, tag="vraw",
                                name=f"vraw{t4}")
                for st in range(4):
                    psv = pgv.tile([P, CV1], f32, tag="v", name=f"v{t4}_{st}")
                    for kt in range(KD):
                        nc.tensor.matmul(psv[:],
                                         xt_t[:, kt, st * P:(st + 1) * P],
                                         wv1_sb[:, kt],
                                         start=(kt == 0), stop=(kt == KD - 1))
                    nc.scalar.copy(vraw[:, st, :], psv[:])

                # squares for the NEXT slab now, so the DVE queue reaches
                # them before this slab's Newton chain (which waits on the
                # sxxc DMA round trip)
                if t4 + 1 < NT4:
                    xsq_tiles[t4 + 1] = emit_xsq(t4 + 1)

                # mu for each token tile from the Sx columns
                for st in range(4):
                    tts = t4 * 4 + st
                    nc.vector.tensor_scalar(mu_c[:, tts:tts + 1],
                                            vraw[:, st, CV:CV + 1],
                                            1.0 / DIM, None, ALU.mult)

                # r = rsqrt(var + eps), slab granularity [P, 4]
                nc.vector.tensor_scalar(ex2e[:, s4], sxxc[:, s4], 1.0 / DIM,
                                        LN_EPS, ALU.mult, ALU.add)
                nc.vector.tensor_tensor(mu2[:, s4], mu_c[:, s4], mu_c[:, s4],
                                        ALU.mult)
                nc.vector.scalar_tensor_tensor(vvar[:, s4], mu2[:, s4], -1.0,
                                               ex2e[:, s4], ALU.mult, ALU.add)
                nc.vector.tensor_scalar(yint[:, s4],
                                        vvar[:, s4].bitcast(i32), 1,
                                        None, ALU.arith_shift_right)
                nc.vector.tensor_tensor(yint[:, s4], magic[:, s4], yint[:, s4],
                                        ALU.subtract)
                y = yint.bitcast(f32)
                for _ in range(3):
                    nc.vector.tensor_tensor(tnw[:, s4], y[:, s4], y[:, s4],
                                            ALU.mult)
                    nc.vector.tensor_tensor(tnw[:, s4], tnw[:, s4], vvar[:, s4],
                                            ALU.mult)
                    nc.vector.tensor_scalar(tnw[:, s4], tnw[:, s4], -0.5, 1.5,
                                            ALU.mult, ALU.add)
                    nc.vector.tensor_tensor(y[:, s4], y[:, s4], tnw[:, s4],
                                            ALU.mult)
                nc.vector.tensor_copy(r_c[:, s4], y[:, s4])
                nc.vector.tensor_scalar(rc1[:, s4], y[:, s4], C1_SCH, None,
                                        ALU.mult)

                # broadcast r along partitions (for the q multiply)
                nc.sync.dma_start(
                    r_dram[0, tsl].rearrange("(o p) -> p o", p=P), r_c[:, s4])
                nc.sync.dma_start(r_bc[:, tsl],
                                  r_dram[0:1, tsl].to_broadcast([P, 512]))

                # v post-scale by r_t (SBUF -> SBUF)
                for st in range(4):
                    tts = t4 * 4 + st
                    vr3 = vraw[:, st, 0:CV].rearrange("p (h d) -> p h d", h=HL)
                    nc.vector.tensor_scalar_mul(
                        vaugs[t4][:, st, :, 0:DH], vr3,
                        r_c[:, tts:tts + 1])

            q_pss = emit_q_mm(NT4 - 1)
            emit_q_fold(NT4 - 1, q_pss)

        # ---------------- Phase 2: attention + interleaved out-proj -------
        with tc.tile_pool(name="pat", bufs=4) as pat, \
             tc.tile_pool(name="pdn", bufs=3) as pdn, \
             tc.tile_pool(name="posb", bufs=3) as posb, \
             tc.tile_pool(name="psc", bufs=2, space="PSUM") as psc, \
             tc.tile_pool(name="ppv", bufs=2, space="PSUM") as ppv, \
             tc.tile_pool(name="pop", bufs=2, space="PSUM") as pop:

            # Deferred-work timeline: every engine op is emitted only at a
            # global slot (= iteration*16 + kt) late enough that its inputs
            # are certainly complete -- a waiting op at the head of an
            # in-order engine queue stalls everything behind it.
            timeline = {}       # slot -> [closure]
            op_queue = []       # (ready_slot, oc, qp)

            def at_slot(slot, fn):
                timeline.setdefault(slot, []).append(fn)

            def outproj_unit():
                _, oc, qp = op_queue.pop(0)
                qsl = slice(qp * 512, (qp + 1) * 512)
                pso = pop.tile([P, 512], f32, tag="op", name=f"op{oc}_{qp}")
                nc.tensor.matmul(pso[:], wout_sb[:, 0, oc * P:(oc + 1) * P],
                                 outT[:, 0, qsl], start=True, stop=False)
                nc.tensor.matmul(pso[:], wout_sb[:, 1, oc * P:(oc + 1) * P],
                                 outT[:, 1, qsl], start=False, stop=True)
                osb = posb.tile([P, 512], f32, tag="osb")
                if oc % 2 == 0:
                    nc.scalar.copy(osb[:], pso[:])
                else:
                    nc.vector.tensor_copy(osb[:], pso[:])
                nc.sync.dma_start(outp_d[oc * P:(oc + 1) * P, qsl], osb[:])

            def sched_tail(i, p_, qp_, dnm2):
                # Denominator path, staged so no engine queue ever holds a
                # waiting op.  The reciprocal runs in all-lane column form
                # (tiny DMA reshape), converts to bf16, returns as a row,
                # and a rank-1 ones-matmul broadcasts it into PSUM.
                qsl_ = slice(qp_ * 512, (qp_ + 1) * 512)
                sA = (i + 1) * KT + 2

                def stage_a():
                    dnr = pdn.tile([1, 1024], f32, tag="dnr")
                    nc.vector.reciprocal_approx_fast(dnr[:], dnm2[:])
                    dnrb = pdn.tile([1, 1024], bf16, tag="dnrb")
                    nc.gpsimd.tensor_copy(dnrb[:], dnr[:])

                    def stage_bc():
                        # rank-1 ones matmuls broadcast the recip rows
                        # across all partitions (full-128 dst, base 0)
                        bcA = pop.tile([P, 512], f32, tag="op",
                                       name=f"bcA{qp_}_{p_}")
                        bcB = pop.tile([P, 512], f32, tag="op",
                                       name=f"bcB{qp_}_{p_}")
                        nc.tensor.matmul(bcA[:], onesb[:],
                                         dnrb[0:1, 0:512],
                                         start=True, stop=True)
                        nc.tensor.matmul(bcB[:], onesb[:],
                                         dnrb[0:1, 512:1024],
                                         start=True, stop=True)

                        def stage_b():
                            nc.vector.tensor_tensor(outT[0:DH, p_, qsl_],
                                                    outT[0:DH, p_, qsl_],
                                                    bcA[0:DH], ALU.mult)
                            nc.vector.tensor_tensor(outT[DH:P, p_, qsl_],
                                                    outT[DH:P, p_, qsl_],
                                                    bcB[DH:P], ALU.mult)
                            if p_ == 1:
                                op_queue.extend((sA + 10 + 2 * oc, oc, qp_)
                                                for oc in range(DIM // P))
                        at_slot(sA + 7, stage_b)
                    at_slot(sA + 4, stage_bc)
                at_slot(sA, stage_a)

            for qp in range(NQ):
                qsl = slice(qp * 512, (qp + 1) * 512)
                for p in range(2):
                    it = qp * 2 + p
                    hA, hB = 2 * p, 2 * p + 1
                    pvA = ppv.tile([DH + 1, 512], f32, tag="pv",
                                   name=f"pvA{qp}_{p}")
                    pvB = ppv.tile([DH + 1, 512], f32, tag="pv",
                                   name=f"pvB{qp}_{p}")
                    # software-pipelined: scores(kt)+exp(kt) issue before
                    # pv(kt-2), so the PE never sits behind an exp wait
                    LAG = 2
                    ets = {}
                    for kt in range(KT + LAG):
                        slot = it * KT + min(kt, KT - 1)
                        if kt < KT:
                            for fn in timeline.pop(slot, []):
                                fn()
                            if (kt in (1, 3, 5, 7, 15) and op_queue
                                    and op_queue[0][0] <= slot):
                                outproj_unit()
                            ksl = slice(kt * P, (kt + 1) * P)
                            ps = psc.tile([P, 2, 512], f32, tag="sc",
                                          name=f"sc{qp}_{p}_{kt}")
                            # two heads on disjoint PE row groups: concurrent
                            kst = kTs[kt // 4]
                            ksl4 = slice((kt % 4) * P, (kt % 4 + 1) * P)
                            nc.tensor.matmul(ps[:, 0],
                                             kst[0:DH, p, ksl4],
                                             qTs[qp][0:DH, p, :],
                                             start=True, stop=True)
                            nc.tensor.matmul(ps[:, 1],
                                             kst[DH:P, p, ksl4],
                                             qTs[qp][DH:P, p, :],
                                             start=True, stop=True)
                            et = pat.tile([P, 2, 512], bf16, tag="et",
                                          name=f"et{qp}_{p}_{kt}")
                            # exact exp on ScalarE for cols [0, SPL)
                            nc.scalar.activation(et[:, :, 0:SPL],
                                                 ps[:, :, 0:SPL],
                                                 FT.Exp, scale=r_c[:, kt:kt + 1])
                            # bf16-Schraudolph on DVE for cols [SPL, 512)
                            nc.vector.tensor_scalar(
                                et[:, :, SPL:].bitcast(i16), ps[:, :, SPL:],
                                rc1[:, kt:kt + 1], C2_SCH, ALU.mult, ALU.add)
                            ets[kt] = et
                        if kt >= LAG:
                            kl = kt - LAG
                            vs = vaugs[kl // 4]
                            et = ets.pop(kl)
                            nc.tensor.matmul(pvA[:], vs[:, kl % 4, hA, :],
                                             et[:, 0], start=(kt == LAG),
                                             stop=(kt == KT + LAG - 1))
                            nc.tensor.matmul(pvB[:], vs[:, kl % 4, hB, :],
                                             et[:, 1], start=(kt == LAG),
                                             stop=(kt == KT + LAG - 1))

                    # unload attention output + denominator rows (frees
                    # psum); explicit ACT/DVE split
                    nc.scalar.copy(outT[0:DH, p, qsl], pvA[0:DH])
                    nc.vector.tensor_copy(outT[DH:P, p, qsl], pvB[0:DH])
                    dnm2 = pdn.tile([1, 1024], f32, tag="dn2")
                    nc.scalar.copy(dnm2[0:1, 0:512], pvA[DH:DH + 1])
                    nc.vector.tensor_copy(dnm2[0:1, 512:1024], pvB[DH:DH + 1])
                    sched_tail(it, p, qp, dnm2)

            # drain the timeline + remaining out-proj units (closures may
            # add further timeline entries while draining)
            while timeline:
                slot = min(timeline)
                for fn in timeline.pop(slot):
                    fn()
            while op_queue:
                outproj_unit()

    nc.compile()
    return nc


def _prep_inputs(x, ln_gamma, ln_beta, w_qkv, w_out, b_out):
    """Host-side sharding/layout prep. Returns list of per-core input maps."""
    x = np.asarray(x, dtype=np.float32)
    ln_gamma = np.asarray(ln_gamma, dtype=np.float32)
    ln_beta = np.asarray(ln_beta, dtype=np.float32)
    w_qkv = np.asarray(w_qkv, dtype=np.float32)
    w_out = np.asarray(w_out, dtype=np.float32)

    wsc = w_qkv.copy()
    wsc[:, :INNER] *= SCALE                      # fold attn scale into q
    wfold = ln_gamma[:, None] * wsc              # fold LN gamma
    # fold the LN mean correction into the weights:
    # (x - mu) @ W' == x @ (W' - colsum(W')/DIM)  since mu = rowmean(x)
    wfold = wfold - wfold.sum(axis=0, keepdims=True) / DIM
    v0 = ln_beta @ wsc
    assert not np.any(v0 != 0.0), "nonzero ln_beta path not implemented"

    wq, wk, wv_all = np.split(wfold, 3, axis=1)

    in_maps = []
    for c in range(8):
        b = c // 4
        hs = (c % 4) * HL * DH
        sl = slice(hs, hs + HL * DH)
        xb = x[b]                                           # [2048, 1024]
        wqk_loc = np.concatenate([wq[:, sl], wk[:, sl]], axis=1)  # [1024, 512]
        wv1_loc = np.concatenate(
            [wv_all[:, sl], np.ones((DIM, 1), np.float32)], axis=1)
        in_maps.append({
            "xt": np.ascontiguousarray(xb.T).astype(_BF16),
            "wqk": np.ascontiguousarray(wqk_loc).astype(_BF16),
            "wv1": np.ascontiguousarray(wv1_loc).astype(_BF16),
            "wout": np.ascontiguousarray(w_out[sl, :]),
        })
    return in_maps


def run(x, ln_gamma, ln_beta, w_qkv, w_out, b_out, trace=False, trace_kwargs=None):
    in_maps = _prep_inputs(x, ln_gamma, ln_beta, w_qkv, w_out, b_out)
    if "nc" not in _CACHE:
        _CACHE["nc"] = _build()
    nc = _CACHE["nc"]
    kwargs = {}
    if trace:
        kwargs = dict(trace=True, trace_cores=[0],
                      stitch_traces=False, **(trace_kwargs or {}))
    res = bass_utils.run_bass_kernel_spmd(
        nc, in_maps, core_ids=list(range(8)), **kwargs)

    b_out = np.asarray(b_out, dtype=np.float32)
    out = np.zeros((B, N, DIM), dtype=np.float32)
    for b in range(B):
        acc = np.zeros((DIM, T), dtype=np.float32)
        for c in range(4 * b, 4 * b + 4):
            acc += res.results[c]["outp"]
        out[b] = acc.T + b_out
    return out, res


def kernel(x, ln_gamma, ln_beta, w_qkv, w_out, b_out):
    out, _ = run(x, ln_gamma, ln_beta, w_qkv, w_out, b_out, trace=False)
    return out


# revision 48
# speedup vs baseline: 1.1439x; 1.1439x over previous
"""Trainium2 Bass kernel for nn_Attention (LN -> QKV -> softmax attn -> out proj).

Sharding: 8 cores; core c handles batch b=c//4 and heads [4*(c%4), 4*(c%4)+4).
Each core computes one partial output [DIM, T] (both head-pair stacks summed
in PSUM); the host sums 4 partials per batch, transposes, adds b_out.

Device pipeline per core (bf16 matmuls, fp32 PSUM accumulate):
  Phase 1 (per 512-token slab): K/Q/V projections from raw x^T with the
    LayerNorm mean correction folded into the weights on the host
    (W'' = W' - colsum(W')/DIM, so x @ W'' == (x - mu) @ W'), plus:
      - Sx rides the V matmul as an appended ones-column in the moving
        operand (psum col CV = row-sum of x) -- no separate stats matmul.
      - Sxx via one DVE square + 1^T xsq matmuls; rsqrt via bit-trick
        Newton on DVE in token-column layout.
      - q scaled by r on psum->sbuf move (fused DVE multiply with a
        DMA-broadcast r row); k left unscaled (r_key rides the exp scale);
        v scaled by r via per-partition tensor_scalar.
  Phase 2 attention, per (512-query chunk, head pair): the two heads of a
    pair live on PE row-groups 0:64 / 64:128, so their K=64 score matmuls
    run CONCURRENTLY on disjoint quadrants (auto tile_position).  exp is
    split: ScalarE does SPL/512 of each tile exactly; DVE does the rest
    with a one-op bf16 Schraudolph (int16 bits = s*C1 + C2, bitcast).
    P@V accumulates with a ones-column appended to v so the softmax
    denominator lands in the same PSUM tile; denominator reciprocal via
    DMA-reshape + fast-reciprocal, broadcast back through DRAM.
  Out-proj: both stacks accumulate into ONE psum tile per (oc, chunk);
    units are interleaved into the next chunk's attention loop.
"""

import contextlib

import numpy as np

import concourse.bass as bass
import concourse.tile as tile
from concourse import bacc, mybir
from concourse import bass_utils

# Problem constants (hardcoded per contract)
B, N, DIM = 2, 2048, 1024
H, DH = 16, 64
INNER = H * DH
LN_EPS = 1e-5
SCALE = DH ** -0.5

# Per-core constants
P = 128
T = N                 # tokens per core (one batch)
TT = T // P           # 16 token tiles of 128
NT4 = T // 512        # 4 token slabs of 512
KD = DIM // P         # 8 contraction tiles
HL = 4                # local heads per core
CQK = 2 * HL * DH     # 512 (q cols + k cols)
CV = HL * DH          # 256 (v cols)
CV1 = CV + 1          # v cols + ones column (Sx)
KT = T // P           # 16 key tiles of 128
NQ = T // 512         # 4 query chunks of 512

# exp split: ScalarE does cols [0, SPL), DVE-Schraudolph does [SPL, 512)
SPL = 296
# bf16 Schraudolph constants: int16 bits = s * C1 + C2, bitcast to bf16
C1_SCH = float(128.0 * np.log2(np.e))
C2_SCH = float(16256.0 + 128.0 * (-0.043))

f32 = mybir.dt.float32
f32r = mybir.dt.float32r
bf16 = mybir.dt.bfloat16
i16 = mybir.dt.int16
i32 = mybir.dt.int32
FT = mybir.ActivationFunctionType
ALU = mybir.AluOpType

import ml_dtypes
_BF16 = np.dtype(ml_dtypes.bfloat16)

_CACHE = {}


def _build():
    nc = bacc.Bacc("TRN2", target_bir_lowering=False, debug=False)

    xt_d = nc.dram_tensor("xt", [DIM, T], bf16, kind="ExternalInput").ap()
    wqk_d = nc.dram_tensor("wqk", [DIM, CQK], bf16, kind="ExternalInput").ap()
    wv1_d = nc.dram_tensor("wv1", [DIM, CV1], bf16, kind="ExternalInput").ap()
    wout_d = nc.dram_tensor("wout", [2 * P, DIM], f32r, kind="ExternalInput").ap()
    outp_d = nc.dram_tensor("outp", [DIM, T], f32, kind="ExternalOutput").ap()

    with tile.TileContext(nc) as tc, contextlib.ExitStack() as ctx:
        pers = ctx.enter_context(tc.tile_pool(name="pers", bufs=1))
        dram = ctx.enter_context(tc.tile_pool(name="dram", bufs=1, space="DRAM"))

        # per-slab tiles: dependency tracking is per-tile, so slab-sliced
        # tiles keep attention from falsely waiting on the last slab's write
        qTs = [pers.tile([P, 2, 512], bf16, name=f"qT{t}") for t in range(NT4)]
        kTs = [pers.tile([P, 2, 512], bf16, name=f"kT{t}") for t in range(NT4)]
        vaugs = [pers.tile([P, 4, HL, DH + 1], bf16, name=f"vaug{t}")
                 for t in range(NT4)]
        outT = pers.tile([P, 2, T], f32r)            # attention out (pair-stacked)
        wout_sb = pers.tile([P, 2, DIM], f32r)
        r_c = pers.tile([P, TT], f32)                # rsqrt(var+eps), token cols
        rc1 = pers.tile([P, TT], f32)                # r * C1_SCH for DVE exp
        r_bc = pers.tile([P, T], f32)                # r broadcast along partitions
        onesb = pers.tile([1, P], bf16)              # denominator-broadcast lhsT
        mu_c = pers.tile([P, TT], f32)
        sxxc = pers.tile([P, TT], f32)
        ex2e = pers.tile([P, TT], f32)
        mu2 = pers.tile([P, TT], f32)
        vvar = pers.tile([P, TT], f32)
        yint = pers.tile([P, TT], i32)
        tnw = pers.tile([P, TT], f32)
        magic = pers.tile([P, TT], i32)
        sxx_row = pers.tile([1, T], f32)

        sxx_dram = dram.tile([1, T], f32)
        r_dram = dram.tile([1, T], f32)
        dnm_dram = dram.tile([2, 2, T], f32)         # [pair, head, tok]
        rdn_dram = dram.tile([2, 2, T], bf16)

        for t in range(NT4):
            nc.vector.memset(vaugs[t][:], 1.0)
        nc.vector.memset(magic[:], 0x5F3759DF)
        nc.vector.memset(onesb[:], 1.0)

        # ---------------- Phase 1: stats + QKV projection -----------------
        with tc.tile_pool(name="pab", bufs=1) as pab, \
             tc.tile_pool(name="pxt", bufs=3) as pxt, \
             tc.tile_pool(name="pxq", bufs=2) as pxq, \
             tc.tile_pool(name="pvr", bufs=2) as pvr, \
             tc.tile_pool(name="pgk", bufs=3, space="PSUM") as pgk, \
             tc.tile_pool(name="pgv", bufs=4, space="PSUM") as pgv, \
             tc.tile_pool(name="pgs", bufs=1, space="PSUM") as pgs:

            def load_xt(t4):
                # one 3D DMA per slab: per-dma_start Sync issue cost is
                # ~600ns, so 8 separate kt loads serialize the Sync queue
                tsl = slice(t4 * 512, (t4 + 1) * 512)
                xt_t = pxt.tile([P, KD, 512], bf16, tag="xt", name=f"xt{t4}")
                nc.sync.dma_start(
                    xt_t[:], xt_d[:, tsl].rearrange("(o p) t -> p o t", p=P))
                return xt_t

            wqk_sb = pab.tile([P, KD, CQK], bf16)
            # k half first: the first matmuls need it
            nc.sync.dma_start(
                wqk_sb[:, :, CQK // 2:],
                wqk_d[:, CQK // 2:].rearrange("(o p) c -> p o c", p=P))
            xt_tiles = {0: load_xt(0)}
            wv1_sb = pab.tile([P, KD, CV1], bf16)
            nc.sync.dma_start(wv1_sb[:], wv1_d.rearrange("(o p) c -> p o c", p=P))
            nc.sync.dma_start(
                wqk_sb[:, :, :CQK // 2],
                wqk_d[:, :CQK // 2].rearrange("(o p) c -> p o c", p=P))
            nc.sync.dma_start(wout_sb[:], wout_d.rearrange("(o p) c -> p o c", p=P))

            ones_col = pab.tile([P, 1], bf16)
            nc.vector.memset(ones_col[:], 1.0)

            # PE warmup spins during initial DMA fill (HAM un-throttle)
            bones = pab.tile([1, 1], bf16)
            nc.vector.memset(bones[:], 1.0)
            brow = pab.tile([1, 64], bf16)
            nc.vector.memset(brow[:], 1.0)
            warm_ps = pgs.tile([1, 512], f32, tag="st", name="warm0")
            for _ in range(60):
                nc.tensor.matmul(warm_ps[0:1, 0:64], bones[:], brow[:],
                                 start=True, stop=True)

            def emit_xsq(t4):
                # squares for Sxx (DVE, bf16 2x rate)
                xsq = pxq.tile([P, KD, 512], bf16, tag="xsq",
                               name=f"xsq{t4}")
                nc.vector.tensor_tensor(xsq[:], xt_tiles[t4][:],
                                        xt_tiles[t4][:], ALU.mult)
                return xsq

            def emit_q_mm(t4):
                # Q projection matmuls for slab t4 (one slab behind K/V:
                # PE work to cover this slab's stats chain)
                xt_t = xt_tiles.pop(t4)
                pss = []
                for g in (0, 1):
                    ps = pgk.tile([P, 512], f32, tag="qk", name=f"q{t4}_{g}")
                    for kt in range(KD):
                        nc.tensor.matmul(ps[:], wqk_sb[:, kt, g * P:(g + 1) * P],
                                         xt_t[:, kt],
                                         start=(kt == 0), stop=(kt == KD - 1))
                    pss.append(ps)
                return pss

            def emit_q_fold(t4, pss):
                # r folded into the psum->sbuf move; emitted after this
                # slab's Newton so the DVE queue head never waits on the
                # r_bc DMA chain
                tsl = slice(t4 * 512, (t4 + 1) * 512)
                for g in (0, 1):
                    nc.vector.tensor_tensor(qTs[t4][:, g, :], pss[g][:],
                                            r_bc[:, tsl], ALU.mult)

            xsq_tiles = {0: emit_xsq(0)}

            for t4 in range(NT4):
                tsl = slice(t4 * 512, (t4 + 1) * 512)
                s4 = slice(t4 * 4, t4 * 4 + 4)
                if t4 + 1 < NT4:
                    xt_tiles[t4 + 1] = load_xt(t4 + 1)
                xt_t = xt_tiles[t4]
                xsq = xsq_tiles.pop(t4)

                # K projections (k stays un-scaled: r_key rides the exp scale)
                for g in (2, 3):
                    ps = pgk.tile([P, 512], f32, tag="qk", name=f"k{t4}_{g}")
                    for kt in range(KD):
                        nc.tensor.matmul(ps[:], wqk_sb[:, kt, g * P:(g + 1) * P],
                                         xt_t[:, kt],
                                         start=(kt == 0), stop=(kt == KD - 1))
                    nc.scalar.copy(kTs[t4][:, g - 2, :], ps[:])

                # Sxx row = 1^T xsq
                ps_st = pgs.tile([1, 512], f32, tag="st", name=f"st{t4}")
                for kt in range(KD):
                    nc.tensor.matmul(ps_st[0:1], ones_col[:], xsq[:, kt],
                                     start=(kt == 0), stop=(kt == KD - 1))
                nc.scalar.copy(sxx_row[0:1, tsl], ps_st[0:1])
                nc.sync.dma_start(sxx_dram[0:1, tsl], sxx_row[0:1, tsl])
                nc.sync.dma_start(
                    sxxc[:, s4],
                    sxx_dram[0, tsl].rearrange("(o p) -> p o", p=P))

                # V projections + Sx ones-column (psum col CV); staged
                # into SBUF via ACT so the psum bank frees immediately
                vraw = pvr.tile([P, 4, CV1], f32, tag="vraw",
                                name=f"vraw{t4}")
                for st in range(4):
                    psv = pgv.tile([P, CV1], f32, tag="v", name=f"v{t4}_{st}")
                    for kt in range(KD):
                        nc.tensor.matmul(psv[:],
                                         xt_t[:, kt, st * P:(st + 1) * P],
                                         wv1_sb[:, kt],
                                         start=(kt == 0), stop=(kt == KD - 1))
                    nc.scalar.copy(vraw[:, st, :], psv[:])

                # Q matmuls for the previous slab keep the PE busy during
                # this slab's stats chain
                q_pss = emit_q_mm(t4 - 1) if t4 >= 1 else None

                # mu for each token tile from the Sx columns
                for st in range(4):
                    tts = t4 * 4 + st
                    nc.vector.tensor_scalar(mu_c[:, tts:tts + 1],
                                            vraw[:, st, CV:CV + 1],
                                            1.0 / DIM, None, ALU.mult)

                # previous slab's q fold + next slab's squares go into the
                # DVE queue BEFORE this slab's DMA-gated Newton chain --
                # their inputs are already resident
                if q_pss is not None:
                    emit_q_fold(t4 - 1, q_pss)
                if t4 + 1 < NT4:
                    xsq_tiles[t4 + 1] = emit_xsq(t4 + 1)

                # r = rsqrt(var + eps), slab granularity [P, 4]
                nc.vector.tensor_scalar(ex2e[:, s4], sxxc[:, s4], 1.0 / DIM,
                                        LN_EPS, ALU.mult, ALU.add)
                nc.vector.tensor_tensor(mu2[:, s4], mu_c[:, s4], mu_c[:, s4],
                                        ALU.mult)
                nc.vector.scalar_tensor_tensor(vvar[:, s4], mu2[:, s4], -1.0,
                                               ex2e[:, s4], ALU.mult, ALU.add)
                nc.vector.tensor_scalar(yint[:, s4],
                                        vvar[:, s4].bitcast(i32), 1,
                                        None, ALU.arith_shift_right)
                nc.vector.tensor_tensor(yint[:, s4], magic[:, s4], yint[:, s4],
                                        ALU.subtract)
                y = yint.bitcast(f32)
                for _ in range(3):
                    nc.vector.tensor_tensor(tnw[:, s4], y[:, s4], y[:, s4],
                                            ALU.mult)
                    nc.vector.tensor_tensor(tnw[:, s4], tnw[:, s4], vvar[:, s4],
                                            ALU.mult)
                    nc.vector.tensor_scalar(tnw[:, s4], tnw[:, s4], -0.5, 1.5,
                                            ALU.mult, ALU.add)
                    nc.vector.tensor_tensor(y[:, s4], y[:, s4], tnw[:, s4],
                                            ALU.mult)
                nc.vector.tensor_copy(r_c[:, s4], y[:, s4])
                nc.vector.tensor_scalar(rc1[:, s4], y[:, s4], C1_SCH, None,
                                        ALU.mult)

                # broadcast r along partitions (for the q multiply)
                nc.sync.dma_start(
                    r_dram[0, tsl].rearrange("(o p) -> p o", p=P), r_c[:, s4])
                nc.sync.dma_start(r_bc[:, tsl],
                                  r_dram[0:1, tsl].to_broadcast([P, 512]))

                # v post-scale by r_t (SBUF -> SBUF)
                for st in range(4):
                    tts = t4 * 4 + st
                    vr3 = vraw[:, st, 0:CV].rearrange("p (h d) -> p h d", h=HL)
                    nc.vector.tensor_scalar_mul(
                        vaugs[t4][:, st, :, 0:DH], vr3,
                        r_c[:, tts:tts + 1])

            q_pss = emit_q_mm(NT4 - 1)
            emit_q_fold(NT4 - 1, q_pss)

        # ---------------- Phase 2: attention + interleaved out-proj -------
        with tc.tile_pool(name="pat", bufs=4) as pat, \
             tc.tile_pool(name="pdn", bufs=3) as pdn, \
             tc.tile_pool(name="posb", bufs=3) as posb, \
             tc.tile_pool(name="psc", bufs=2, space="PSUM") as psc, \
             tc.tile_pool(name="ppv", bufs=2, space="PSUM") as ppv, \
             tc.tile_pool(name="pop", bufs=2, space="PSUM") as pop:

            # Deferred-work timeline: every engine op is emitted only at a
            # global slot (= iteration*16 + kt) late enough that its inputs
            # are certainly complete -- a waiting op at the head of an
            # in-order engine queue stalls everything behind it.
            timeline = {}       # slot -> [closure]
            op_queue = []       # (ready_slot, oc, qp)

            def at_slot(slot, fn):
                timeline.setdefault(slot, []).append(fn)

            def outproj_unit():
                _, oc, qp = op_queue.pop(0)
                qsl = slice(qp * 512, (qp + 1) * 512)
                pso = pop.tile([P, 512], f32, tag="op", name=f"op{oc}_{qp}")
                nc.tensor.matmul(pso[:], wout_sb[:, 0, oc * P:(oc + 1) * P],
                                 outT[:, 0, qsl], start=True, stop=False)
                nc.tensor.matmul(pso[:], wout_sb[:, 1, oc * P:(oc + 1) * P],
                                 outT[:, 1, qsl], start=False, stop=True)
                osb = posb.tile([P, 512], f32, tag="osb")
                if oc % 2 == 0:
                    nc.scalar.copy(osb[:], pso[:])
                else:
                    nc.vector.tensor_copy(osb[:], pso[:])
                nc.sync.dma_start(outp_d[oc * P:(oc + 1) * P, qsl], osb[:])

            def sched_tail(i, p_, qp_, dnm2):
                # Denominator path, staged so no engine queue ever holds a
                # waiting op.  The reciprocal runs in all-lane column form
                # (tiny DMA reshape), converts to bf16, returns as a row,
                # and a rank-1 ones-matmul broadcasts it into PSUM.
                qsl_ = slice(qp_ * 512, (qp_ + 1) * 512)
                sA = (i + 1) * KT + 2

                def stage_a():
                    dnr = pdn.tile([1, 1024], f32, tag="dnr")
                    nc.vector.reciprocal_approx_fast(dnr[:], dnm2[:])
                    dnrb = pdn.tile([1, 1024], bf16, tag="dnrb")
                    nc.gpsimd.tensor_copy(dnrb[:], dnr[:])

                    def stage_bc():
                        # rank-1 ones matmuls broadcast the recip rows
                        # across all partitions (full-128 dst, base 0)
                        bcA = pop.tile([P, 512], f32, tag="op",
                                       name=f"bcA{qp_}_{p_}")
                        bcB = pop.tile([P, 512], f32, tag="op",
                                       name=f"bcB{qp_}_{p_}")
                        nc.tensor.matmul(bcA[:], onesb[:],
                                         dnrb[0:1, 0:512],
                                         start=True, stop=True)
                        nc.tensor.matmul(bcB[:], onesb[:],
                                         dnrb[0:1, 512:1024],
                                         start=True, stop=True)

                        def stage_b():
                            nc.vector.tensor_tensor(outT[0:DH, p_, qsl_],
                                                    outT[0:DH, p_, qsl_],
                                                    bcA[0:DH], ALU.mult)
                            nc.vector.tensor_tensor(outT[DH:P, p_, qsl_],
                                                    outT[DH:P, p_, qsl_],
                                                    bcB[DH:P], ALU.mult)
                            if p_ == 1:
                                op_queue.extend((sA + 10 + 2 * oc, oc, qp_)
                                                for oc in range(DIM // P))
                        at_slot(sA + 7, stage_b)
                    at_slot(sA + 4, stage_bc)
                at_slot(sA, stage_a)

            for qp in range(NQ):
                qsl = slice(qp * 512, (qp + 1) * 512)
                for p in range(2):
                    it = qp * 2 + p
                    hA, hB = 2 * p, 2 * p + 1
                    pvA = ppv.tile([DH + 1, 512], f32, tag="pv",
                                   name=f"pvA{qp}_{p}")
                    pvB = ppv.tile([DH + 1, 512], f32, tag="pv",
                                   name=f"pvB{qp}_{p}")
                    # software-pipelined: scores(kt)+exp(kt) issue before
                    # pv(kt-2), so the PE never sits behind an exp wait
                    LAG = 2
                    ets = {}
                    for kt in range(KT + LAG):
                        slot = it * KT + min(kt, KT - 1)
                        if kt < KT:
                            for fn in timeline.pop(slot, []):
                                fn()
                            if (kt in (1, 3, 5, 7, 15) and op_queue
                                    and op_queue[0][0] <= slot):
                                outproj_unit()
                            ksl = slice(kt * P, (kt + 1) * P)
                            ps = psc.tile([P, 2, 512], f32, tag="sc",
                                          name=f"sc{qp}_{p}_{kt}")
                            # two heads on disjoint PE row groups: concurrent
                            kst = kTs[kt // 4]
                            ksl4 = slice((kt % 4) * P, (kt % 4 + 1) * P)
                            nc.tensor.matmul(ps[:, 0],
                                             kst[0:DH, p, ksl4],
                                             qTs[qp][0:DH, p, :],
                                             start=True, stop=True)
                            nc.tensor.matmul(ps[:, 1],
                                             kst[DH:P, p, ksl4],
                                             qTs[qp][DH:P, p, :],
                                             start=True, stop=True)
                            et = pat.tile([P, 2, 512], bf16, tag="et",
                                          name=f"et{qp}_{p}_{kt}")
                            # exact exp on ScalarE for cols [0, SPL)
                            nc.scalar.activation(et[:, :, 0:SPL],
                                                 ps[:, :, 0:SPL],
                                                 FT.Exp, scale=r_c[:, kt:kt + 1])
                            # bf16-Schraudolph on DVE for cols [SPL, 512)
                            nc.vector.tensor_scalar(
                                et[:, :, SPL:].bitcast(i16), ps[:, :, SPL:],
                                rc1[:, kt:kt + 1], C2_SCH, ALU.mult, ALU.add)
                            ets[kt] = et
                        if kt >= LAG:
                            kl = kt - LAG
                            vs = vaugs[kl // 4]
                            et = ets.pop(kl)
                            nc.tensor.matmul(pvA[:], vs[:, kl % 4, hA, :],
                                             et[:, 0], start=(kt == LAG),
                                             stop=(kt == KT + LAG - 1))
                            nc.tensor.matmul(pvB[:], vs[:, kl % 4, hB, :],
                                             et[:, 1], start=(kt == LAG),
                                             stop=(kt == KT + LAG - 1))

                    # unload attention output + denominator rows (frees
                    # psum); explicit ACT/DVE split
                    nc.scalar.copy(outT[0:DH, p, qsl], pvA[0:DH])
                    nc.vector.tensor_copy(outT[DH:P, p, qsl], pvB[0:DH])
                    dnm2 = pdn.tile([1, 1024], f32, tag="dn2")
                    nc.scalar.copy(dnm2[0:1, 0:512], pvA[DH:DH + 1])
                    nc.vector.tensor_copy(dnm2[0:1, 512:1024], pvB[DH:DH + 1])
                    sched_tail(it, p, qp, dnm2)

            # drain the timeline + remaining out-proj units (closures may
            # add further timeline entries while draining)
            while timeline:
                slot = min(timeline)
                for fn in timeline.pop(slot):
                    fn()
            while op_queue:
                outproj_unit()

    nc.compile()
    return nc


def _prep_inputs(x, ln_gamma, ln_beta, w_qkv, w_out, b_out):
    """Host-side sharding/layout prep. Returns list of per-core input maps."""
    x = np.asarray(x, dtype=np.float32)
    ln_gamma = np.asarray(ln_gamma, dtype=np.float32)
    ln_beta = np.asarray(ln_beta, dtype=np.float32)
    w_qkv = np.asarray(w_qkv, dtype=np.float32)
    w_out = np.asarray(w_out, dtype=np.float32)

    wsc = w_qkv.copy()
    wsc[:, :INNER] *= SCALE                      # fold attn scale into q
    wfold = ln_gamma[:, None] * wsc              # fold LN gamma
    # fold the LN mean correction into the weights:
    # (x - mu) @ W' == x @ (W' - colsum(W')/DIM)  since mu = rowmean(x)
    wfold = wfold - wfold.sum(axis=0, keepdims=True) / DIM
    v0 = ln_beta @ wsc
    assert not np.any(v0 != 0.0), "nonzero ln_beta path not implemented"

    wq, wk, wv_all = np.split(wfold, 3, axis=1)

    in_maps = []
    for c in range(8):
        b = c // 4
        hs = (c % 4) * HL * DH
        sl = slice(hs, hs + HL * DH)
        xb = x[b]                                           # [2048, 1024]
        wqk_loc = np.concatenate([wq[:, sl], wk[:, sl]], axis=1)  # [1024, 512]
        wv1_loc = np.concatenate(
            [wv_all[:, sl], np.ones((DIM, 1), np.float32)], axis=1)
        in_maps.append({
            "xt": np.ascontiguousarray(xb.T).astype(_BF16),
            "wqk": np.ascontiguousarray(wqk_loc).astype(_BF16),
            "wv1": np.ascontiguousarray(wv1_loc).astype(_BF16),
            "wout": np.ascontiguousarray(w_out[sl, :]),
        })
    return in_maps


def run(x, ln_gamma, ln_beta, w_qkv, w_out, b_out, trace=False, trace_kwargs=None):
    in_maps = _prep_inputs(x, ln_gamma, ln_beta, w_qkv, w_out, b_out)
    if "nc" not in _CACHE:
        _CACHE["nc"] = _build()
    nc = _CACHE["nc"]
    kwargs = {}
    if trace:
        kwargs = dict(trace=True, trace_cores=[0],
                      stitch_traces=False, **(trace_kwargs or {}))
    res = bass_utils.run_bass_kernel_spmd(
        nc, in_maps, core_ids=list(range(8)), **kwargs)

    b_out = np.asarray(b_out, dtype=np.float32)
    out = np.zeros((B, N, DIM), dtype=np.float32)
    for b in range(B):
        acc = np.zeros((DIM, T), dtype=np.float32)
        for c in range(4 * b, 4 * b + 4):
            acc += res.results[c]["outp"]
        out[b] = acc.T + b_out
    return out, res


def kernel(x, ln_gamma, ln_beta, w_qkv, w_out, b_out):
    out, _ = run(x, ln_gamma, ln_beta, w_qkv, w_out, b_out, trace=False)
    return out
